# revision 14
# baseline (speedup 1.0000x reference)
"""GAT (PyG GATConv + Linear) on 8 Trainium2 NeuronCores.

Strategy (edge-parallel / 1D node partition, per the sharding hint):
  - Nodes are range-partitioned over the 8 cores by destination id
    (12500 dst nodes per core); each core holds its edge shard.
  - Kernel-1 (device): F_T = W_ext.T @ x.T where W_ext = W @ [I | As | Ad]
    is assembled on device from host-index-shuffled matrices; one bf16
    matmul per 512-node chunk produces [h(c-major 16) | a_src(8) | a_dst(8)]
    per node, emitted in fp16.
  - Host: pure index shuffling -- the per-edge join F[src_e] is materialized
    on the host into each core's dst-sorted CSR slot stream (degree-sorted
    128-node tiles, rectangular padding; pad slots get a_src = -60000 so that
    exp(lrelu(...)) == 0 exactly).  The host performs no model arithmetic.
  - Kernel-2 (device): per-tile s = a_src + a_dst (DVE fp16), leaky-relu and
    exp on the Scalar engine, q = p * h (DVE fp16 2x mode), then a pairwise
    in-place fold tree replaces the 1x-rate tensor_reduce for the segment
    sums; softmax normalization and the 16->2 linear head run in fp32.
  All per-edge tensors are fp16 (half the HBM traffic of fp32); exp() gets a
  free bias of -5 (a global logit shift that cancels in the softmax) so all
  magnitudes stay in fp16 normal range.  Numerics validated against the fp32
  reference at rel-err ~2e-3 (gate is 2e-2).
"""
import os
import sys
import time

for _p in ("/opt/trn_rl_repo", "/root/.axon_site/_ro/trn_rl_repo"):
    if os.path.isdir(_p) and _p not in sys.path:
        sys.path.append(_p)

import numpy as np
import ml_dtypes

F16 = np.float16

N_NODES = 100000
N_CORES = 8
IN_F = 128
HEADS = 8
OUT_C = 2
HC = HEADS * OUT_C          # 16
NEG_SLOPE = 0.2
NODES_PER_CORE = N_NODES // N_CORES   # 12500
P = 128
NT = 100                              # tiles (25 groups of 4)
GRP = 10
NT_K1 = 25                            # kernel-1 node chunks of 512
CHUNK = 512
NP = NT * P                           # 12800
PAD_ASRC = -60000.0
EXP_SHIFT = -5.0  # global logit shift (cancels in softmax); keeps exp() in fp16 range


# ----------------------------------------------------------------- host prep
def _build_shards(edge_index):
    src = np.asarray(edge_index[0], dtype=np.int64)
    dst = np.asarray(edge_index[1], dtype=np.int64)
    loops = np.arange(N_NODES, dtype=np.int64)
    src = np.concatenate([src, loops])
    dst = np.concatenate([dst, loops])

    core = dst // NODES_PER_CORE
    deg = np.bincount(dst, minlength=N_NODES)

    perms, srcs_by_core, dloc_by_core = [], [], []
    ptab_per_core = np.zeros((N_CORES, NT), np.int64)
    for c in range(N_CORES):
        lo = c * NODES_PER_CORE
        d = deg[lo:lo + NODES_PER_CORE]
        order = np.argsort(-d, kind="stable")
        perm = np.full(NP, -1, np.int64)
        perm[:NODES_PER_CORE] = np.arange(lo, lo + NODES_PER_CORE)[order]
        perms.append(perm)
        dd_pad = np.concatenate([d[order], np.zeros(NP - NODES_PER_CORE, np.int64)])
        ptab_per_core[c] = dd_pad.reshape(NT, P).max(axis=1)
        rank_of_node = np.empty(NODES_PER_CORE, np.int64)
        rank_of_node[order] = np.arange(NODES_PER_CORE)
        m = core == c
        srcs_by_core.append(src[m])
        dloc_by_core.append(rank_of_node[dst[m] - lo])

    ptab = np.maximum(ptab_per_core.max(axis=0), 1)
    ptab = np.repeat(ptab.reshape(NT // GRP, GRP).max(axis=1), GRP)
    S = int((ptab * P).sum())
    tilebase = np.concatenate([[0], np.cumsum(ptab * P)[:-1]])

    slot_srcs = []
    for c in range(N_CORES):
        s = np.full(S, -1, np.int64)
        dloc = dloc_by_core[c]
        esrc = srcs_by_core[c]
        order = np.argsort(dloc, kind="stable")
        dloc_s = dloc[order]
        esrc_s = esrc[order]
        _, cnt = np.unique(dloc_s, return_counts=True)
        j = np.arange(len(dloc_s)) - np.repeat(np.cumsum(cnt) - cnt, cnt)
        ts = dloc_s // P
        ps = dloc_s % P
        s[tilebase[ts] + ps * ptab[ts] + j] = esrc_s
        slot_srcs.append(s)

    return {"perms": perms, "ptab": ptab, "tilebase": tilebase, "S": S,
            "slot_srcs": slot_srcs}


def _e_matrix():
    """[I16(c-major) | As | Ad] column layout selectors (host: pure indexing).

    E has shape [16, 32]; W_ext = W @ E.  Columns 0:8 pick (h, c=0) rows,
    8:16 pick (h, c=1), 16:24 hold att_src (a_src weights), 24:32 att_dst.
    The att values are placed by indexing, never combined arithmetically.
    """
    return None  # built in kernel() with actual att weights


# ------------------------------------------------------------- bass kernels
def _build_kernel1(body_reps=1):
    import concourse.bacc as bacc
    import concourse.tile as tile
    import concourse.mybir as mybir

    nc = bacc.Bacc("TRN2", target_bir_lowering=False, debug=False,
                   enable_asserts=True, num_devices=N_CORES)
    xT = nc.dram_tensor("xT", [P, NP], mybir.dt.float32, kind="ExternalInput").ap()
    Wt16 = nc.dram_tensor("Wt16", [HC, P], mybir.dt.float32,
                          kind="ExternalInput").ap()
    E = nc.dram_tensor("E", [HC, 32], mybir.dt.float32, kind="ExternalInput").ap()
    F = nc.dram_tensor("F", [32, NP], mybir.dt.float16, kind="ExternalOutput").ap()

    PIECE = 2560                       # 5 DMA pieces, 5 matmuls each
    with tile.TileContext(nc) as tc:
        with (
            tc.tile_pool(name="sbuf", bufs=1) as pool,
            tc.tile_pool(name="xf", bufs=3) as xfpool,
            tc.tile_pool(name="xh", bufs=2) as xhpool,
            tc.tile_pool(name="psumw", bufs=1, space="PSUM") as psumw,
            tc.tile_pool(name="psum", bufs=6, space="PSUM") as psum,
        ):
            Wt16_sb = pool.tile([HC, P], mybir.dt.float32)
            E_sb = pool.tile([HC, 32], mybir.dt.float32)
            Wext = pool.tile([P, 32], mybir.dt.float16)
            Fbuf = pool.tile([32, NT_K1, CHUNK], mybir.dt.float16)

            nc.sync.dma_start(out=Wt16_sb[:], in_=Wt16[:])
            nc.sync.dma_start(out=E_sb[:], in_=E[:])

            # W_ext = W @ [Icmaj | As | Ad]  (fp32 matmul, cast to fp16)
            pw = psumw.tile([P, 32], mybir.dt.float32, tag="pw")
            nc.tensor.matmul(out=pw[:], lhsT=Wt16_sb[:], rhs=E_sb[:],
                             start=True, stop=True)
            nc.vector.tensor_copy(out=Wext[:], in_=pw[:])

            for _rep in range(body_reps):
                for piece in range(NP // PIECE):
                    xf = xfpool.tile([P, PIECE], mybir.dt.float32, tag="xf")
                    xh = xhpool.tile([P, PIECE], mybir.dt.float16, tag="xh")
                    o = piece * PIECE
                    nc.sync.dma_start(out=xf[:], in_=xT[:, o:o + PIECE])
                    nc.vector.tensor_copy(out=xh[:], in_=xf[:])  # fp32->fp16
                    for k in range(PIECE // CHUNK):
                        t = piece * (PIECE // CHUNK) + k
                        pc = psum.tile([32, CHUNK], mybir.dt.float32, tag="pc")
                        nc.tensor.matmul(out=pc[:],
                                         lhsT=Wext[:],
                                         rhs=xh[:, k * CHUNK:(k + 1) * CHUNK],
                                         start=True, stop=True)
                        nc.scalar.copy(out=Fbuf[:, t, :], in_=pc[:])

            nc.sync.dma_start(out=F.rearrange("f (t n) -> f t n", t=NT_K1),
                              in_=Fbuf[:])
    nc.compile()
    return nc


def _build_kernel2(ptab, tilebase, S, body_reps=1):
    import concourse.bacc as bacc
    import concourse.tile as tile
    import concourse.mybir as mybir

    ptab = [int(v) for v in ptab]
    tilebase = [int(v) for v in tilebase]
    nc = bacc.Bacc("TRN2", target_bir_lowering=False, debug=False,
                   enable_asserts=True, num_devices=N_CORES)
    SF = nc.dram_tensor("SF", [S, 24], mybir.dt.float16,
                        kind="ExternalInput").ap()
    AD = nc.dram_tensor("AD", [P, NT * HEADS], mybir.dt.float16,
                        kind="ExternalInput").ap()
    brep = nc.dram_tensor("brep", [P, HC], mybir.dt.float32,
                          kind="ExternalInput").ap()
    w0 = nc.dram_tensor("w0", [P, HC], mybir.dt.float32, kind="ExternalInput").ap()
    w1 = nc.dram_tensor("w1", [P, HC], mybir.dt.float32, kind="ExternalInput").ap()
    bfc = nc.dram_tensor("bfc", [P, 2], mybir.dt.float32, kind="ExternalInput").ap()
    OUT = nc.dram_tensor("OUT", [P, NT * 2], mybir.dt.float32,
                         kind="ExternalOutput").ap()

    pmax = max(ptab)
    with tile.TileContext(nc) as tc:
        with tc.tile_pool(name="sbuf", bufs=1) as cpool, \
             tc.tile_pool(name="feat", bufs=4) as fpool:
            AD_sb = cpool.tile([P, NT, HEADS], mybir.dt.float16)
            brep_sb = cpool.tile([P, HC], mybir.dt.float32)
            w0_sb = cpool.tile([P, HC], mybir.dt.float32)
            w1_sb = cpool.tile([P, HC], mybir.dt.float32)
            bfc_sb = cpool.tile([P, 2], mybir.dt.float32)
            SQ = cpool.tile([P, NT, 24], mybir.dt.float16)
            shift_sb = cpool.tile([P, 1], mybir.dt.float32)
            rec = cpool.tile([P, NT, HEADS], mybir.dt.float32, tag="rec")
            agg = cpool.tile([P, NT, HC], mybir.dt.float32)
            outb = cpool.tile([P, NT, 2], mybir.dt.float32)

            nc.vector.memset(shift_sb[:], EXP_SHIFT)
            nc.sync.dma_start(out=AD_sb[:],
                              in_=AD.rearrange("p (t h) -> p t h", t=NT))
            nc.sync.dma_start(out=brep_sb[:], in_=brep[:])
            nc.sync.dma_start(out=w0_sb[:], in_=w0[:])
            nc.sync.dma_start(out=w1_sb[:], in_=w1[:])
            nc.sync.dma_start(out=bfc_sb[:], in_=bfc[:])

            for _rep in range(body_reps):
                order_g = sorted(range(NT // GRP), key=lambda g: ptab[g * GRP])
                for g in order_g:
                    t0 = g * GRP
                    pt = ptab[t0]            # equal within a group
                    base = tilebase[t0]
                    ns = GRP * pt            # slots per partition in group
                    feat = fpool.tile([P, GRP * pmax * 24], mybir.dt.float16,
                                      tag="feat")
                    nc.sync.dma_start(
                        out=feat[:, :ns * 24].rearrange("p (t x) -> p t x", t=GRP),
                        in_=SF[base:base + GRP * P * pt, :].rearrange(
                            "(t p j) f -> p t (j f)", t=GRP, p=P),
                    )
                    fv = feat[:, :ns * 24].rearrange("p (t j f) -> p t j f",
                                                     t=GRP, f=24)
                    # s = a_src + a_dst (fp16 2x, in place)
                    nc.vector.tensor_tensor(
                        out=fv[:, :, :, 16:24],
                        in0=fv[:, :, :, 16:24],
                        in1=AD_sb[:, t0:t0 + GRP, None, :].broadcast_to(
                            [P, GRP, pt, HEADS]),
                        op=mybir.AluOpType.add)
                    # p = exp(lrelu(s))  -- both on the Scalar engine, in place
                    nc.scalar.activation(out=fv[:, :, :, 16:24],
                                         in_=fv[:, :, :, 16:24],
                                         func=mybir.ActivationFunctionType.Prelu,
                                         alpha=NEG_SLOPE)
                    nc.scalar.activation(out=fv[:, :, :, 16:24],
                                         in_=fv[:, :, :, 16:24],
                                         func=mybir.ActivationFunctionType.Exp,
                                         bias=shift_sb[:])
                    # q = p * h in place (h is c-major: [c=2, h=8])
                    nc.vector.tensor_tensor(
                        out=fv[:, :, :, 0:16].rearrange("p t j (c h) -> p t j c h",
                                                        c=2),
                        in0=fv[:, :, :, 0:16].rearrange("p t j (c h) -> p t j c h",
                                                        c=2),
                        in1=fv[:, :, :, None, 16:24].broadcast_to(
                            [P, GRP, pt, 2, HEADS]),
                        op=mybir.AluOpType.mult)
                    # pairwise fold tree over j: [q|p] summed per dst
                    cur = pt
                    while cur > 2:
                        k = cur // 2
                        nc.vector.tensor_tensor(
                            out=fv[:, :, 0:k, :],
                            in0=fv[:, :, 0:k, :],
                            in1=fv[:, :, cur - k:cur, :],
                            op=mybir.AluOpType.add)
                        cur -= k
                    if cur == 2:
                        nc.vector.tensor_tensor(
                            out=SQ[:, t0:t0 + GRP, :],
                            in0=fv[:, :, 0, :],
                            in1=fv[:, :, 1, :],
                            op=mybir.AluOpType.add)
                    else:  # pt == 1 tile group
                        nc.vector.tensor_copy(out=SQ[:, t0:t0 + GRP, :],
                                              in_=fv[:, :, 0, :])

            # ---- softmax normalization + bias + 16->2 head (fp32 tail)
            # Two halves: tiles [50:100] complete after the first 5 (small-pt,
            # high-tile-index) groups, so that half overlaps group compute.
            tmp = cpool.tile([P, NT, HC], mybir.dt.float32, tag="tmp")
            def _tail(lo, hi):
                n = hi - lo
                nc.vector.reciprocal(out=rec[:, lo:hi], in_=SQ[:, lo:hi, 16:24])
                nc.vector.tensor_tensor(
                    out=agg[:, lo:hi].rearrange("p t (c h) -> p t c h", c=2),
                    in0=SQ[:, lo:hi, 0:16].rearrange("p t (c h) -> p t c h", c=2),
                    in1=rec[:, lo:hi, None, :].broadcast_to([P, n, 2, HEADS]),
                    op=mybir.AluOpType.mult)
                nc.vector.tensor_tensor(
                    out=agg[:, lo:hi], in0=agg[:, lo:hi],
                    in1=brep_sb[:, None, :].broadcast_to([P, n, HC]),
                    op=mybir.AluOpType.add)
                for wsb, col in ((w0_sb, 0), (w1_sb, 1)):
                    nc.vector.tensor_tensor(
                        out=tmp[:, lo:hi], in0=agg[:, lo:hi],
                        in1=wsb[:, None, :].broadcast_to([P, n, HC]),
                        op=mybir.AluOpType.mult)
                    nc.vector.tensor_reduce(out=outb[:, lo:hi, col],
                                            in_=tmp[:, lo:hi],
                                            axis=mybir.AxisListType.X,
                                            op=mybir.AluOpType.add)
                nc.vector.tensor_tensor(
                    out=outb[:, lo:hi], in0=outb[:, lo:hi],
                    in1=bfc_sb[:, None, :].broadcast_to([P, n, 2]),
                    op=mybir.AluOpType.add)
                nc.sync.dma_start(
                    out=OUT.rearrange("p (t c) -> p t c", t=NT)[:, lo:hi],
                    in_=outb[:, lo:hi])
            _tail(NT // 2, NT)
            _tail(0, NT // 2)
    nc.compile()
    return nc


# ------------------------------------------------------------------ runner
class _Runner:
    """Reusable jitted shard_map executor for a compiled Bacc kernel."""

    def __init__(self, nc, in_maps):
        import jax
        from jax.sharding import Mesh, PartitionSpec, NamedSharding
        from jax.experimental.shard_map import shard_map
        from concourse import bass2jax, mybir

        bass2jax.install_neuronx_cc_hook()
        partition_name = (nc.partition_id_tensor.name
                          if nc.partition_id_tensor else None)
        in_names, out_names, out_avals, zero_outs = [], [], [], []
        for alloc in nc.m.functions[0].allocations:
            if not isinstance(alloc, mybir.MemoryLocationSet):
                continue
            name = alloc.memorylocations[0].name
            if alloc.kind == "ExternalInput":
                if name != partition_name:
                    in_names.append(name)
            elif alloc.kind == "ExternalOutput":
                shape = tuple(alloc.tensor_shape)
                dtype = mybir.dt.np(alloc.dtype)
                out_names.append(name)
                out_avals.append(jax.core.ShapedArray(shape, dtype))
                zero_outs.append(np.zeros(shape, dtype))
        n_params = len(in_names)
        all_in = list(in_names) + list(out_names)
        if partition_name is not None:
            all_in.append(partition_name)

        def _body(*args):
            operands = list(args)
            if partition_name is not None:
                operands.append(bass2jax.partition_id_tensor())
            return tuple(bass2jax._bass_exec_p.bind(
                *operands, out_avals=tuple(out_avals), in_names=tuple(all_in),
                out_names=tuple(out_names), lowering_input_output_aliases=(),
                sim_require_finite=True, sim_require_nnan=True, nc=nc))

        devices = jax.devices()[:N_CORES]
        mesh = Mesh(np.asarray(devices), ("core",))
        specs = (PartitionSpec("core"),)
        self._fn = jax.jit(
            shard_map(_body, mesh=mesh,
                      in_specs=specs * (n_params + len(out_avals)),
                      out_specs=specs * len(out_avals), check_rep=False),
            keep_unused=True)
        per_core = [[np.asarray(m[name]) for name in in_names] for m in in_maps]
        concat_in = [np.concatenate([per_core[c][i] for c in range(N_CORES)], axis=0)
                     for i in range(n_params)]
        concat_zero = [np.zeros((N_CORES * z.shape[0], *z.shape[1:]), z.dtype)
                       for z in zero_outs]
        sh = NamedSharding(mesh, PartitionSpec("core"))
        self._args = [jax.device_put(a, sh) for a in concat_in + concat_zero]
        self._out_names = out_names
        self._out_avals = out_avals
        self._jax = jax

    def run(self):
        outs = self._fn(*self._args)
        return [
            {name: np.asarray(outs[i]).reshape(N_CORES, *self._out_avals[i].shape)[c]
             for i, name in enumerate(self._out_names)}
            for c in range(N_CORES)
        ]

    def time(self, iters=8, warmup=2):
        for _ in range(warmup):
            self._jax.block_until_ready(self._fn(*self._args))
        walls = []
        for _ in range(iters):
            t0 = time.perf_counter()
            self._jax.block_until_ready(self._fn(*self._args))
            walls.append(time.perf_counter() - t0)
        return min(walls)


# ----------------------------------------------------------- input builders
_CMAJ = np.array([0, 2, 4, 6, 8, 10, 12, 14, 1, 3, 5, 7, 9, 11, 13, 15])


def _build_maps1(x, W, att_src, att_dst):
    """Per-core kernel-1 inputs.  All host work is index shuffling."""
    E = np.zeros((HC, 32), np.float32)
    E[_CMAJ, np.arange(16)] = 1.0                       # c-major identity
    for h in range(HEADS):
        for c in range(OUT_C):
            E[h * OUT_C + c, 16 + h] = att_src[h, c]    # placement, no math
            E[h * OUT_C + c, 24 + h] = att_dst[h, c]
    Wt16 = np.ascontiguousarray(W.T)                    # [16, 128]
    maps1 = []
    for c in range(N_CORES):
        xT = np.zeros((P, NP), np.float32)
        xT[:, :NODES_PER_CORE] = x[c * NODES_PER_CORE:(c + 1) * NODES_PER_CORE].T
        maps1.append({"xT": xT, "Wt16": Wt16, "E": E})
    return maps1


def _build_maps2(shards, F_full, bias_gat, W_fc, b_fc):
    """Per-core kernel-2 inputs from the kernel-1 output F_full [N, 32] bf16."""
    S = shards["S"]
    brep = np.tile(bias_gat[_CMAJ].reshape(1, HC), (P, 1)).astype(np.float32)
    w0 = np.tile(W_fc[_CMAJ, 0].reshape(1, HC), (P, 1)).astype(np.float32)
    w1 = np.tile(W_fc[_CMAJ, 1].reshape(1, HC), (P, 1)).astype(np.float32)
    bfcr = np.tile(b_fc.reshape(1, 2), (P, 1)).astype(np.float32)
    maps2 = []
    for c in range(N_CORES):
        ssrc = shards["slot_srcs"][c]
        perm = shards["perms"][c]
        SF = np.zeros((S, 24), F16)
        SF[:, 16:24] = F16(PAD_ASRC)
        real = ssrc >= 0
        SF[real] = F_full[ssrc[real], 0:24]
        AD = np.zeros((NP, HEADS), F16)
        pr = perm >= 0
        AD[pr] = F_full[perm[pr], 24:32]
        AD = np.ascontiguousarray(
            AD.reshape(NT, P, HEADS).transpose(1, 0, 2)).reshape(P, NT * HEADS)
        maps2.append({"SF": SF, "AD": AD, "brep": brep, "w0": w0, "w1": w1,
                      "bfc": bfcr})
    return maps2


def _gather_F(res1):
    """Assemble F_full [N_NODES, 32] bf16 from per-core F [32, NP] outputs."""
    parts = []
    for c in range(N_CORES):
        Ft = res1[c]["F"]                    # [32, NP] bf16
        parts.append(np.ascontiguousarray(Ft[:, :NODES_PER_CORE].T))
    return np.concatenate(parts, axis=0)


# ------------------------------------------------------------------- kernel
def kernel(**inputs):
    x = np.asarray(inputs["x"], np.float32)
    edge_index = np.asarray(inputs["edge_index"])
    W = np.asarray(inputs["W"], np.float32)
    att_src = np.asarray(inputs["att_src"], np.float32)
    att_dst = np.asarray(inputs["att_dst"], np.float32)
    bias_gat = np.asarray(inputs["bias_gat"], np.float32)
    W_fc = np.asarray(inputs["W_fc"], np.float32)
    b_fc = np.asarray(inputs["b_fc"], np.float32)
    # edge_attr intentionally ignored (GATConv built without edge_dim).

    shards = _build_shards(edge_index)

    def _run_retrying(build_nc, maps, attempts=3):
        last = None
        for i in range(attempts):
            try:
                return _Runner(build_nc(), maps).run()
            except Exception as e:  # transient device desync seen on this setup
                last = e
                time.sleep(2.0)
        raise last

    # ---- kernel 1: F_T = [h(c-major) | a_src | a_dst] per node, bf16
    maps1 = _build_maps1(x, W, att_src, att_dst)
    res1 = _run_retrying(_build_kernel1, maps1)
    F_full = _gather_F(res1)

    # ---- host shuffle: materialize per-core dst-CSR slot streams (bf16)
    maps2 = _build_maps2(shards, F_full, bias_gat, W_fc, b_fc)

    # ---- kernel 2: edge math + aggregation + head
    res2 = _run_retrying(
        lambda: _build_kernel2(shards["ptab"], shards["tilebase"], shards["S"]),
        maps2)

    out = np.zeros((N_NODES, 2), np.float32)
    for c in range(N_CORES):
        perm = shards["perms"][c]
        pr = perm >= 0
        o = res2[c]["OUT"].reshape(P, NT, 2).transpose(1, 0, 2).reshape(NP, 2)
        out[perm[pr]] = o[pr]
    return out


# revision 19
# speedup vs baseline: 1.0072x; 1.0072x over previous
"""GAT (PyG GATConv + Linear) on 8 Trainium2 NeuronCores.

Strategy (edge-parallel / 1D node partition, per the sharding hint):
  - Nodes are range-partitioned over the 8 cores by destination id
    (12500 dst nodes per core); each core holds its edge shard.
  - Kernel-1 (device): F_T = W_ext.T @ x.T where W_ext = W @ [I | As | Ad]
    is assembled on device from host-index-shuffled matrices; one bf16
    matmul per 512-node chunk produces [h(c-major 16) | a_src(8) | a_dst(8)]
    per node, emitted in fp16.
  - Host: pure index shuffling -- the per-edge join F[src_e] is materialized
    on the host into each core's dst-sorted CSR slot stream (degree-sorted
    128-node tiles, rectangular padding; pad slots get a_src = -60000 so that
    exp(lrelu(...)) == 0 exactly).  The host performs no model arithmetic.
  - Kernel-2 (device): per-tile s = a_src + a_dst (DVE fp16), leaky-relu and
    exp on the Scalar engine, q = p * h (DVE fp16 2x mode), then a pairwise
    in-place fold tree replaces the 1x-rate tensor_reduce for the segment
    sums; softmax normalization and the 16->2 linear head run in fp32.
  All per-edge tensors are fp16 (half the HBM traffic of fp32); exp() gets a
  free bias of -5 (a global logit shift that cancels in the softmax) so all
  magnitudes stay in fp16 normal range.  Numerics validated against the fp32
  reference at rel-err ~2e-3 (gate is 2e-2).
"""
import os
import sys
import time

for _p in ("/opt/trn_rl_repo", "/root/.axon_site/_ro/trn_rl_repo"):
    if os.path.isdir(_p) and _p not in sys.path:
        sys.path.append(_p)

import numpy as np

F16 = np.float16

N_NODES = 100000
N_CORES = 8
IN_F = 128
HEADS = 8
OUT_C = 2
HC = HEADS * OUT_C          # 16
NEG_SLOPE = 0.2
NODES_PER_CORE = N_NODES // N_CORES   # 12500
P = 128
NT = 100                              # tiles (25 groups of 4)
GRP = 10
NT_K1 = 25                            # kernel-1 node chunks of 512
CHUNK = 512
NP = NT * P                           # 12800
PAD_ASRC = -60000.0
EXP_SHIFT = -5.0  # global logit shift (cancels in softmax); keeps exp() in fp16 range


# ----------------------------------------------------------------- host prep
def _build_shards(edge_index):
    src = np.asarray(edge_index[0], dtype=np.int64)
    dst = np.asarray(edge_index[1], dtype=np.int64)
    loops = np.arange(N_NODES, dtype=np.int64)
    src = np.concatenate([src, loops])
    dst = np.concatenate([dst, loops])

    core = dst // NODES_PER_CORE
    deg = np.bincount(dst, minlength=N_NODES)

    perms, srcs_by_core, dloc_by_core = [], [], []
    ptab_per_core = np.zeros((N_CORES, NT), np.int64)
    for c in range(N_CORES):
        lo = c * NODES_PER_CORE
        d = deg[lo:lo + NODES_PER_CORE]
        order = np.argsort(-d, kind="stable")
        perm = np.full(NP, -1, np.int64)
        perm[:NODES_PER_CORE] = np.arange(lo, lo + NODES_PER_CORE)[order]
        perms.append(perm)
        dd_pad = np.concatenate([d[order], np.zeros(NP - NODES_PER_CORE, np.int64)])
        ptab_per_core[c] = dd_pad.reshape(NT, P).max(axis=1)
        rank_of_node = np.empty(NODES_PER_CORE, np.int64)
        rank_of_node[order] = np.arange(NODES_PER_CORE)
        m = core == c
        srcs_by_core.append(src[m])
        dloc_by_core.append(rank_of_node[dst[m] - lo])

    ptab = np.maximum(ptab_per_core.max(axis=0), 1)
    ptab = np.repeat(ptab.reshape(NT // GRP, GRP).max(axis=1), GRP)
    S = int((ptab * P).sum())
    tilebase = np.concatenate([[0], np.cumsum(ptab * P)[:-1]])

    slot_srcs = []
    for c in range(N_CORES):
        s = np.full(S, -1, np.int64)
        dloc = dloc_by_core[c]
        esrc = srcs_by_core[c]
        order = np.argsort(dloc, kind="stable")
        dloc_s = dloc[order]
        esrc_s = esrc[order]
        _, cnt = np.unique(dloc_s, return_counts=True)
        j = np.arange(len(dloc_s)) - np.repeat(np.cumsum(cnt) - cnt, cnt)
        ts = dloc_s // P
        ps = dloc_s % P
        s[tilebase[ts] + ps * ptab[ts] + j] = esrc_s
        slot_srcs.append(s)

    return {"perms": perms, "ptab": ptab, "tilebase": tilebase, "S": S,
            "slot_srcs": slot_srcs}


# ------------------------------------------------------------- bass kernels
def _build_kernel1(body_reps=1):
    import concourse.bacc as bacc
    import concourse.tile as tile
    import concourse.mybir as mybir

    nc = bacc.Bacc("TRN2", target_bir_lowering=False, debug=False,
                   enable_asserts=True, num_devices=N_CORES)
    xT = nc.dram_tensor("xT", [P, NP], mybir.dt.float32, kind="ExternalInput").ap()
    Wt16 = nc.dram_tensor("Wt16", [HC, P], mybir.dt.float32,
                          kind="ExternalInput").ap()
    E = nc.dram_tensor("E", [HC, 32], mybir.dt.float32, kind="ExternalInput").ap()
    F = nc.dram_tensor("F", [32, NP], mybir.dt.float16, kind="ExternalOutput").ap()

    PIECE = 2560                       # 5 DMA pieces, 5 matmuls each
    with tile.TileContext(nc) as tc:
        with (
            tc.tile_pool(name="sbuf", bufs=1) as pool,
            tc.tile_pool(name="xf", bufs=5) as xfpool,
            tc.tile_pool(name="xh", bufs=3) as xhpool,
            tc.tile_pool(name="psumw", bufs=1, space="PSUM") as psumw,
            tc.tile_pool(name="psum", bufs=6, space="PSUM") as psum,
        ):
            Wt16_sb = pool.tile([HC, P], mybir.dt.float32)
            E_sb = pool.tile([HC, 32], mybir.dt.float32)
            Wext = pool.tile([P, 32], mybir.dt.float16)
            Fbuf = pool.tile([32, NT_K1, CHUNK], mybir.dt.float16)

            nc.sync.dma_start(out=Wt16_sb[:], in_=Wt16[:])
            nc.sync.dma_start(out=E_sb[:], in_=E[:])

            # W_ext = W @ [Icmaj | As | Ad]  (fp32 matmul, cast to fp16)
            pw = psumw.tile([P, 32], mybir.dt.float32, tag="pw")
            nc.tensor.matmul(out=pw[:], lhsT=Wt16_sb[:], rhs=E_sb[:],
                             start=True, stop=True)
            nc.vector.tensor_copy(out=Wext[:], in_=pw[:])

            for _rep in range(body_reps):
                for piece in range(NP // PIECE):
                    xf = xfpool.tile([P, PIECE], mybir.dt.float32, tag="xf")
                    xh = xhpool.tile([P, PIECE], mybir.dt.float16, tag="xh")
                    o = piece * PIECE
                    nc.sync.dma_start(out=xf[:], in_=xT[:, o:o + PIECE])
                    nc.vector.tensor_copy(out=xh[:], in_=xf[:])  # fp32->fp16
                    for k in range(PIECE // CHUNK):
                        t = piece * (PIECE // CHUNK) + k
                        pc = psum.tile([32, CHUNK], mybir.dt.float32, tag="pc")
                        nc.tensor.matmul(out=pc[:],
                                         lhsT=Wext[:],
                                         rhs=xh[:, k * CHUNK:(k + 1) * CHUNK],
                                         start=True, stop=True)
                        nc.scalar.copy(out=Fbuf[:, t, :], in_=pc[:])

            nc.sync.dma_start(out=F.rearrange("f (t n) -> f t n", t=NT_K1),
                              in_=Fbuf[:])
    nc.compile()
    return nc


def _build_kernel2(ptab, tilebase, S, body_reps=1):
    import concourse.bacc as bacc
    import concourse.tile as tile
    import concourse.mybir as mybir

    ptab = [int(v) for v in ptab]
    tilebase = [int(v) for v in tilebase]
    nc = bacc.Bacc("TRN2", target_bir_lowering=False, debug=False,
                   enable_asserts=True, num_devices=N_CORES)
    SF = nc.dram_tensor("SF", [S, 24], mybir.dt.float16,
                        kind="ExternalInput").ap()
    AD = nc.dram_tensor("AD", [P, NT * HEADS], mybir.dt.float16,
                        kind="ExternalInput").ap()
    brep = nc.dram_tensor("brep", [P, HC], mybir.dt.float32,
                          kind="ExternalInput").ap()
    w0 = nc.dram_tensor("w0", [P, HC], mybir.dt.float32, kind="ExternalInput").ap()
    w1 = nc.dram_tensor("w1", [P, HC], mybir.dt.float32, kind="ExternalInput").ap()
    bfc = nc.dram_tensor("bfc", [P, 2], mybir.dt.float32, kind="ExternalInput").ap()
    OUT = nc.dram_tensor("OUT", [P, NT * 2], mybir.dt.float32,
                         kind="ExternalOutput").ap()

    pmax = max(ptab)
    with tile.TileContext(nc) as tc:
        with tc.tile_pool(name="sbuf", bufs=1) as cpool, \
             tc.tile_pool(name="feat", bufs=5) as fpool:
            AD_sb = cpool.tile([P, NT, HEADS], mybir.dt.float16)
            brep_sb = cpool.tile([P, HC], mybir.dt.float32)
            w0_sb = cpool.tile([P, HC], mybir.dt.float32)
            w1_sb = cpool.tile([P, HC], mybir.dt.float32)
            bfc_sb = cpool.tile([P, 2], mybir.dt.float32)
            SQ = cpool.tile([P, NT, 24], mybir.dt.float16)
            shift_sb = cpool.tile([P, 1], mybir.dt.float32)
            rec = cpool.tile([P, NT, HEADS], mybir.dt.float32, tag="rec")
            agg = cpool.tile([P, NT, HC], mybir.dt.float32)
            outb = cpool.tile([P, NT, 2], mybir.dt.float32)

            nc.vector.memset(shift_sb[:], EXP_SHIFT)
            nc.sync.dma_start(out=AD_sb[:],
                              in_=AD.rearrange("p (t h) -> p t h", t=NT))
            nc.sync.dma_start(out=brep_sb[:], in_=brep[:])
            nc.sync.dma_start(out=w0_sb[:], in_=w0[:])
            nc.sync.dma_start(out=w1_sb[:], in_=w1[:])
            nc.sync.dma_start(out=bfc_sb[:], in_=bfc[:])

            for _rep in range(body_reps):
                order_g = sorted(range(NT // GRP), key=lambda g: ptab[g * GRP])
                for g in order_g:
                    t0 = g * GRP
                    pt = ptab[t0]            # equal within a group
                    base = tilebase[t0]
                    ns = GRP * pt            # slots per partition in group
                    feat = fpool.tile([P, GRP * pmax * 24], mybir.dt.float16,
                                      tag="feat")
                    nc.sync.dma_start(
                        out=feat[:, :ns * 24].rearrange("p (t x) -> p t x", t=GRP),
                        in_=SF[base:base + GRP * P * pt, :].rearrange(
                            "(t p j) f -> p t (j f)", t=GRP, p=P),
                    )
                    fv = feat[:, :ns * 24].rearrange("p (t j f) -> p t j f",
                                                     t=GRP, f=24)
                    # s = a_src + a_dst (fp16 2x, in place)
                    nc.vector.tensor_tensor(
                        out=fv[:, :, :, 16:24],
                        in0=fv[:, :, :, 16:24],
                        in1=AD_sb[:, t0:t0 + GRP, None, :].broadcast_to(
                            [P, GRP, pt, HEADS]),
                        op=mybir.AluOpType.add)
                    # p = exp(lrelu(s))  -- both on the Scalar engine, in place
                    nc.scalar.activation(out=fv[:, :, :, 16:24],
                                         in_=fv[:, :, :, 16:24],
                                         func=mybir.ActivationFunctionType.Prelu,
                                         alpha=NEG_SLOPE)
                    nc.scalar.activation(out=fv[:, :, :, 16:24],
                                         in_=fv[:, :, :, 16:24],
                                         func=mybir.ActivationFunctionType.Exp,
                                         bias=shift_sb[:])
                    # q = p * h in place (h is c-major: [c=2, h=8])
                    nc.vector.tensor_tensor(
                        out=fv[:, :, :, 0:16].rearrange("p t j (c h) -> p t j c h",
                                                        c=2),
                        in0=fv[:, :, :, 0:16].rearrange("p t j (c h) -> p t j c h",
                                                        c=2),
                        in1=fv[:, :, :, None, 16:24].broadcast_to(
                            [P, GRP, pt, 2, HEADS]),
                        op=mybir.AluOpType.mult)
                    # pairwise fold tree over j: [q|p] summed per dst
                    cur = pt
                    while cur > 2:
                        k = cur // 2
                        nc.vector.tensor_tensor(
                            out=fv[:, :, 0:k, :],
                            in0=fv[:, :, 0:k, :],
                            in1=fv[:, :, cur - k:cur, :],
                            op=mybir.AluOpType.add)
                        cur -= k
                    if cur == 2:
                        nc.vector.tensor_tensor(
                            out=SQ[:, t0:t0 + GRP, :],
                            in0=fv[:, :, 0, :],
                            in1=fv[:, :, 1, :],
                            op=mybir.AluOpType.add)
                    else:  # pt == 1 tile group
                        nc.vector.tensor_copy(out=SQ[:, t0:t0 + GRP, :],
                                              in_=fv[:, :, 0, :])

            # ---- softmax normalization + bias + 16->2 head (fp32 tail)
            # Two halves: tiles [50:100] complete after the first 5 (small-pt,
            # high-tile-index) groups, so that half overlaps group compute.
            tmp = cpool.tile([P, NT, HC], mybir.dt.float32, tag="tmp")
            def _tail(lo, hi):
                n = hi - lo
                nc.vector.reciprocal(out=rec[:, lo:hi], in_=SQ[:, lo:hi, 16:24])
                nc.vector.tensor_tensor(
                    out=agg[:, lo:hi].rearrange("p t (c h) -> p t c h", c=2),
                    in0=SQ[:, lo:hi, 0:16].rearrange("p t (c h) -> p t c h", c=2),
                    in1=rec[:, lo:hi, None, :].broadcast_to([P, n, 2, HEADS]),
                    op=mybir.AluOpType.mult)
                nc.vector.tensor_tensor(
                    out=agg[:, lo:hi], in0=agg[:, lo:hi],
                    in1=brep_sb[:, None, :].broadcast_to([P, n, HC]),
                    op=mybir.AluOpType.add)
                for wsb, col in ((w0_sb, 0), (w1_sb, 1)):
                    nc.vector.tensor_tensor(
                        out=tmp[:, lo:hi], in0=agg[:, lo:hi],
                        in1=wsb[:, None, :].broadcast_to([P, n, HC]),
                        op=mybir.AluOpType.mult)
                    nc.vector.tensor_reduce(out=outb[:, lo:hi, col],
                                            in_=tmp[:, lo:hi],
                                            axis=mybir.AxisListType.X,
                                            op=mybir.AluOpType.add)
                nc.vector.tensor_tensor(
                    out=outb[:, lo:hi], in0=outb[:, lo:hi],
                    in1=bfc_sb[:, None, :].broadcast_to([P, n, 2]),
                    op=mybir.AluOpType.add)
                nc.sync.dma_start(
                    out=OUT.rearrange("p (t c) -> p t c", t=NT)[:, lo:hi],
                    in_=outb[:, lo:hi])
            _tail(NT // 2, NT)
            _tail(0, NT // 2)
    nc.compile()
    return nc


# ------------------------------------------------------------------ runner
class _Runner:
    """Reusable jitted shard_map executor for a compiled Bacc kernel."""

    def __init__(self, nc, in_maps):
        import jax
        from jax.sharding import Mesh, PartitionSpec, NamedSharding
        from jax.experimental.shard_map import shard_map
        from concourse import bass2jax, mybir

        bass2jax.install_neuronx_cc_hook()
        partition_name = (nc.partition_id_tensor.name
                          if nc.partition_id_tensor else None)
        in_names, out_names, out_avals, zero_outs = [], [], [], []
        for alloc in nc.m.functions[0].allocations:
            if not isinstance(alloc, mybir.MemoryLocationSet):
                continue
            name = alloc.memorylocations[0].name
            if alloc.kind == "ExternalInput":
                if name != partition_name:
                    in_names.append(name)
            elif alloc.kind == "ExternalOutput":
                shape = tuple(alloc.tensor_shape)
                dtype = mybir.dt.np(alloc.dtype)
                out_names.append(name)
                out_avals.append(jax.core.ShapedArray(shape, dtype))
                zero_outs.append(np.zeros(shape, dtype))
        n_params = len(in_names)
        all_in = list(in_names) + list(out_names)
        if partition_name is not None:
            all_in.append(partition_name)

        def _body(*args):
            operands = list(args)
            if partition_name is not None:
                operands.append(bass2jax.partition_id_tensor())
            return tuple(bass2jax._bass_exec_p.bind(
                *operands, out_avals=tuple(out_avals), in_names=tuple(all_in),
                out_names=tuple(out_names), lowering_input_output_aliases=(),
                sim_require_finite=True, sim_require_nnan=True, nc=nc))

        devices = jax.devices()[:N_CORES]
        mesh = Mesh(np.asarray(devices), ("core",))
        specs = (PartitionSpec("core"),)
        self._fn = jax.jit(
            shard_map(_body, mesh=mesh,
                      in_specs=specs * (n_params + len(out_avals)),
                      out_specs=specs * len(out_avals), check_rep=False),
            keep_unused=True)
        per_core = [[np.asarray(m[name]) for name in in_names] for m in in_maps]
        concat_in = [np.concatenate([per_core[c][i] for c in range(N_CORES)], axis=0)
                     for i in range(n_params)]
        concat_zero = [np.zeros((N_CORES * z.shape[0], *z.shape[1:]), z.dtype)
                       for z in zero_outs]
        sh = NamedSharding(mesh, PartitionSpec("core"))
        self._args = [jax.device_put(a, sh) for a in concat_in + concat_zero]
        self._out_names = out_names
        self._out_avals = out_avals
        self._jax = jax

    def run(self):
        outs = self._fn(*self._args)
        return [
            {name: np.asarray(outs[i]).reshape(N_CORES, *self._out_avals[i].shape)[c]
             for i, name in enumerate(self._out_names)}
            for c in range(N_CORES)
        ]

    def time(self, iters=8, warmup=2):
        for _ in range(warmup):
            self._jax.block_until_ready(self._fn(*self._args))
        walls = []
        for _ in range(iters):
            t0 = time.perf_counter()
            self._jax.block_until_ready(self._fn(*self._args))
            walls.append(time.perf_counter() - t0)
        return min(walls)


# ----------------------------------------------------------- input builders
_CMAJ = np.array([0, 2, 4, 6, 8, 10, 12, 14, 1, 3, 5, 7, 9, 11, 13, 15])


def _build_maps1(x, W, att_src, att_dst):
    """Per-core kernel-1 inputs.  All host work is index shuffling."""
    E = np.zeros((HC, 32), np.float32)
    E[_CMAJ, np.arange(16)] = 1.0                       # c-major identity
    for h in range(HEADS):
        for c in range(OUT_C):
            E[h * OUT_C + c, 16 + h] = att_src[h, c]    # placement, no math
            E[h * OUT_C + c, 24 + h] = att_dst[h, c]
    Wt16 = np.ascontiguousarray(W.T)                    # [16, 128]
    maps1 = []
    for c in range(N_CORES):
        xT = np.zeros((P, NP), np.float32)
        xT[:, :NODES_PER_CORE] = x[c * NODES_PER_CORE:(c + 1) * NODES_PER_CORE].T
        maps1.append({"xT": xT, "Wt16": Wt16, "E": E})
    return maps1


def _build_maps2(shards, F_full, bias_gat, W_fc, b_fc):
    """Per-core kernel-2 inputs from the kernel-1 output F_full [N, 32] bf16."""
    S = shards["S"]
    brep = np.tile(bias_gat[_CMAJ].reshape(1, HC), (P, 1)).astype(np.float32)
    w0 = np.tile(W_fc[_CMAJ, 0].reshape(1, HC), (P, 1)).astype(np.float32)
    w1 = np.tile(W_fc[_CMAJ, 1].reshape(1, HC), (P, 1)).astype(np.float32)
    bfcr = np.tile(b_fc.reshape(1, 2), (P, 1)).astype(np.float32)
    maps2 = []
    for c in range(N_CORES):
        ssrc = shards["slot_srcs"][c]
        perm = shards["perms"][c]
        SF = np.zeros((S, 24), F16)
        SF[:, 16:24] = F16(PAD_ASRC)
        real = ssrc >= 0
        SF[real] = F_full[ssrc[real], 0:24]
        AD = np.zeros((NP, HEADS), F16)
        pr = perm >= 0
        AD[pr] = F_full[perm[pr], 24:32]
        AD = np.ascontiguousarray(
            AD.reshape(NT, P, HEADS).transpose(1, 0, 2)).reshape(P, NT * HEADS)
        maps2.append({"SF": SF, "AD": AD, "brep": brep, "w0": w0, "w1": w1,
                      "bfc": bfcr})
    return maps2


def _gather_F(res1):
    """Assemble F_full [N_NODES, 32] bf16 from per-core F [32, NP] outputs."""
    parts = []
    for c in range(N_CORES):
        Ft = res1[c]["F"]                    # [32, NP] bf16
        parts.append(np.ascontiguousarray(Ft[:, :NODES_PER_CORE].T))
    return np.concatenate(parts, axis=0)


# ------------------------------------------------------------------- kernel
def kernel(**inputs):
    x = np.asarray(inputs["x"], np.float32)
    edge_index = np.asarray(inputs["edge_index"])
    W = np.asarray(inputs["W"], np.float32)
    att_src = np.asarray(inputs["att_src"], np.float32)
    att_dst = np.asarray(inputs["att_dst"], np.float32)
    bias_gat = np.asarray(inputs["bias_gat"], np.float32)
    W_fc = np.asarray(inputs["W_fc"], np.float32)
    b_fc = np.asarray(inputs["b_fc"], np.float32)
    # edge_attr intentionally ignored (GATConv built without edge_dim).

    shards = _build_shards(edge_index)

    def _run_retrying(build_nc, maps, attempts=3):
        last = None
        for i in range(attempts):
            try:
                return _Runner(build_nc(), maps).run()
            except Exception as e:  # transient device desync seen on this setup
                last = e
                time.sleep(2.0)
        raise last

    # ---- kernel 1: F_T = [h(c-major) | a_src | a_dst] per node, bf16
    maps1 = _build_maps1(x, W, att_src, att_dst)
    res1 = _run_retrying(_build_kernel1, maps1)
    F_full = _gather_F(res1)

    # ---- host shuffle: materialize per-core dst-CSR slot streams (bf16)
    maps2 = _build_maps2(shards, F_full, bias_gat, W_fc, b_fc)

    # ---- kernel 2: edge math + aggregation + head
    res2 = _run_retrying(
        lambda: _build_kernel2(shards["ptab"], shards["tilebase"], shards["S"]),
        maps2)

    out = np.zeros((N_NODES, 2), np.float32)
    for c in range(N_CORES):
        perm = shards["perms"][c]
        pr = perm >= 0
        o = res2[c]["OUT"].reshape(P, NT, 2).transpose(1, 0, 2).reshape(NP, 2)
        out[perm[pr]] = o[pr]
    return out


# revision 22
# speedup vs baseline: 1.0092x; 1.0020x over previous
"""GAT (PyG GATConv + Linear) on 8 Trainium2 NeuronCores.

Strategy (edge-parallel / 1D node partition, per the sharding hint):
  - Nodes are range-partitioned over the 8 cores by destination id
    (12500 dst nodes per core); each core holds its edge shard.
  - Kernel-1 (device): F_T = W_ext.T @ x.T where W_ext = W @ [I | As | Ad]
    is assembled on device from host-index-shuffled matrices; one fp16
    matmul per 512-node chunk produces [h(c-major 16) | a_src(8) | a_dst(8)]
    per node, emitted in fp16.
  - Host: pure index shuffling -- the per-edge join F[src_e] is materialized
    on the host into each core's dst-sorted CSR slot stream (degree-sorted
    128-node tiles, rectangular padding; pad slots get a_src = -60000 so that
    exp(lrelu(...)) == 0 exactly).  The host performs no model arithmetic.
  - Kernel-2 (device): per-tile s = a_src + a_dst (DVE fp16), leaky-relu and
    exp on the Scalar engine, q = p * h (DVE fp16 2x mode), then a pairwise
    in-place fold tree replaces the 1x-rate tensor_reduce for the segment
    sums; softmax normalization and the 16->2 linear head run in fp32.
  All per-edge tensors are fp16 (half the HBM traffic of fp32); exp() gets a
  free bias of -5 (a global logit shift that cancels in the softmax) so all
  magnitudes stay in fp16 normal range.  Numerics validated against the fp32
  reference at rel-err ~2e-3 (gate is 2e-2).
"""
import os
import sys
import time

for _p in ("/opt/trn_rl_repo", "/root/.axon_site/_ro/trn_rl_repo"):
    if os.path.isdir(_p) and _p not in sys.path:
        sys.path.append(_p)

import numpy as np

F16 = np.float16

N_NODES = 100000
N_CORES = 8
IN_F = 128
HEADS = 8
OUT_C = 2
HC = HEADS * OUT_C          # 16
NEG_SLOPE = 0.2
NODES_PER_CORE = N_NODES // N_CORES   # 12500
P = 128
NT = 100                              # tiles (25 groups of 4)
GRP = 10
NT_K1 = 25                            # kernel-1 node chunks of 512
CHUNK = 512
NP = NT * P                           # 12800
PAD_ASRC = -60000.0
EXP_SHIFT = -5.0  # global logit shift (cancels in softmax); keeps exp() in fp16 range


# ----------------------------------------------------------------- host prep
def _build_shards(edge_index):
    src = np.asarray(edge_index[0], dtype=np.int64)
    dst = np.asarray(edge_index[1], dtype=np.int64)
    loops = np.arange(N_NODES, dtype=np.int64)
    src = np.concatenate([src, loops])
    dst = np.concatenate([dst, loops])

    core = dst // NODES_PER_CORE
    deg = np.bincount(dst, minlength=N_NODES)

    perms, srcs_by_core, dloc_by_core = [], [], []
    ptab_per_core = np.zeros((N_CORES, NT), np.int64)
    for c in range(N_CORES):
        lo = c * NODES_PER_CORE
        d = deg[lo:lo + NODES_PER_CORE]
        order = np.argsort(-d, kind="stable")
        perm = np.full(NP, -1, np.int64)
        perm[:NODES_PER_CORE] = np.arange(lo, lo + NODES_PER_CORE)[order]
        perms.append(perm)
        dd_pad = np.concatenate([d[order], np.zeros(NP - NODES_PER_CORE, np.int64)])
        ptab_per_core[c] = dd_pad.reshape(NT, P).max(axis=1)
        rank_of_node = np.empty(NODES_PER_CORE, np.int64)
        rank_of_node[order] = np.arange(NODES_PER_CORE)
        m = core == c
        srcs_by_core.append(src[m])
        dloc_by_core.append(rank_of_node[dst[m] - lo])

    ptab = np.maximum(ptab_per_core.max(axis=0), 1)
    ptab = np.repeat(ptab.reshape(NT // GRP, GRP).max(axis=1), GRP)
    S = int((ptab * P).sum())
    tilebase = np.concatenate([[0], np.cumsum(ptab * P)[:-1]])

    slot_srcs = []
    for c in range(N_CORES):
        s = np.full(S, -1, np.int64)
        dloc = dloc_by_core[c]
        esrc = srcs_by_core[c]
        order = np.argsort(dloc, kind="stable")
        dloc_s = dloc[order]
        esrc_s = esrc[order]
        _, cnt = np.unique(dloc_s, return_counts=True)
        j = np.arange(len(dloc_s)) - np.repeat(np.cumsum(cnt) - cnt, cnt)
        ts = dloc_s // P
        ps = dloc_s % P
        s[tilebase[ts] + ps * ptab[ts] + j] = esrc_s
        slot_srcs.append(s)

    return {"perms": perms, "ptab": ptab, "tilebase": tilebase, "S": S,
            "slot_srcs": slot_srcs}


# ------------------------------------------------------------- bass kernels
def _build_kernel1(body_reps=1):
    import concourse.bacc as bacc
    import concourse.tile as tile
    import concourse.mybir as mybir

    nc = bacc.Bacc("TRN2", target_bir_lowering=False, debug=False,
                   enable_asserts=True, num_devices=N_CORES)
    xT = nc.dram_tensor("xT", [P, NP], mybir.dt.float32, kind="ExternalInput").ap()
    Wt16 = nc.dram_tensor("Wt16", [HC, P], mybir.dt.float32,
                          kind="ExternalInput").ap()
    E = nc.dram_tensor("E", [HC, 32], mybir.dt.float32, kind="ExternalInput").ap()
    F = nc.dram_tensor("F", [32, NP], mybir.dt.float16, kind="ExternalOutput").ap()

    PIECE = 2560                       # 5 DMA pieces, 5 matmuls each
    with tile.TileContext(nc) as tc:
        with (
            tc.tile_pool(name="sbuf", bufs=1) as pool,
            tc.tile_pool(name="xf", bufs=5) as xfpool,
            tc.tile_pool(name="xh", bufs=3) as xhpool,
            tc.tile_pool(name="psumw", bufs=1, space="PSUM") as psumw,
            tc.tile_pool(name="psum", bufs=6, space="PSUM") as psum,
        ):
            Wt16_sb = pool.tile([HC, P], mybir.dt.float32)
            E_sb = pool.tile([HC, 32], mybir.dt.float32)
            Wext = pool.tile([P, 32], mybir.dt.float16)
            Fbuf = pool.tile([32, NT_K1, CHUNK], mybir.dt.float16)

            nc.sync.dma_start(out=Wt16_sb[:], in_=Wt16[:])
            nc.sync.dma_start(out=E_sb[:], in_=E[:])

            # W_ext = W @ [Icmaj | As | Ad]  (fp32 matmul, cast to fp16)
            pw = psumw.tile([P, 32], mybir.dt.float32, tag="pw")
            nc.tensor.matmul(out=pw[:], lhsT=Wt16_sb[:], rhs=E_sb[:],
                             start=True, stop=True)
            nc.vector.tensor_copy(out=Wext[:], in_=pw[:])

            for _rep in range(body_reps):
                for piece in range(NP // PIECE):
                    xf = xfpool.tile([P, PIECE], mybir.dt.float32, tag="xf")
                    xh = xhpool.tile([P, PIECE], mybir.dt.float16, tag="xh")
                    o = piece * PIECE
                    nc.sync.dma_start(out=xf[:], in_=xT[:, o:o + PIECE])
                    nc.vector.tensor_copy(out=xh[:], in_=xf[:])  # fp32->fp16
                    for k in range(PIECE // CHUNK):
                        t = piece * (PIECE // CHUNK) + k
                        pc = psum.tile([32, CHUNK], mybir.dt.float32, tag="pc")
                        nc.tensor.matmul(out=pc[:],
                                         lhsT=Wext[:],
                                         rhs=xh[:, k * CHUNK:(k + 1) * CHUNK],
                                         start=True, stop=True)
                        nc.scalar.copy(out=Fbuf[:, t, :], in_=pc[:])

            nc.sync.dma_start(out=F.rearrange("f (t n) -> f t n", t=NT_K1),
                              in_=Fbuf[:])
    nc.compile()
    return nc


def _build_kernel2(ptab, tilebase, S, body_reps=1):
    import concourse.bacc as bacc
    import concourse.tile as tile
    import concourse.mybir as mybir

    ptab = [int(v) for v in ptab]
    tilebase = [int(v) for v in tilebase]
    nc = bacc.Bacc("TRN2", target_bir_lowering=False, debug=False,
                   enable_asserts=True, num_devices=N_CORES)
    SF = nc.dram_tensor("SF", [S, 24], mybir.dt.float16,
                        kind="ExternalInput").ap()
    AD = nc.dram_tensor("AD", [P, NT * HEADS], mybir.dt.float16,
                        kind="ExternalInput").ap()
    brep = nc.dram_tensor("brep", [P, HC], mybir.dt.float32,
                          kind="ExternalInput").ap()
    w0 = nc.dram_tensor("w0", [P, HC], mybir.dt.float32, kind="ExternalInput").ap()
    w1 = nc.dram_tensor("w1", [P, HC], mybir.dt.float32, kind="ExternalInput").ap()
    bfc = nc.dram_tensor("bfc", [P, 2], mybir.dt.float32, kind="ExternalInput").ap()
    OUT = nc.dram_tensor("OUT", [P, NT * 2], mybir.dt.float32,
                         kind="ExternalOutput").ap()

    pmax = max(ptab)
    with tile.TileContext(nc) as tc:
        with tc.tile_pool(name="sbuf", bufs=1) as cpool, \
             tc.tile_pool(name="feat", bufs=5) as fpool:
            AD_sb = cpool.tile([P, NT, HEADS], mybir.dt.float16)
            brep_sb = cpool.tile([P, HC], mybir.dt.float32)
            w0_sb = cpool.tile([P, HC], mybir.dt.float32)
            w1_sb = cpool.tile([P, HC], mybir.dt.float32)
            bfc_sb = cpool.tile([P, 2], mybir.dt.float32)
            SQ = cpool.tile([P, NT, 24], mybir.dt.float16)
            shift_sb = cpool.tile([P, 1], mybir.dt.float32)
            rec = cpool.tile([P, NT, HEADS], mybir.dt.float32, tag="rec")
            agg = cpool.tile([P, NT, HC], mybir.dt.float32)
            outb = cpool.tile([P, NT, 2], mybir.dt.float32)

            nc.vector.memset(shift_sb[:], EXP_SHIFT)
            nc.sync.dma_start(out=AD_sb[:],
                              in_=AD.rearrange("p (t h) -> p t h", t=NT))
            nc.sync.dma_start(out=brep_sb[:], in_=brep[:])
            nc.sync.dma_start(out=w0_sb[:], in_=w0[:])
            nc.sync.dma_start(out=w1_sb[:], in_=w1[:])
            nc.sync.dma_start(out=bfc_sb[:], in_=bfc[:])

            for _rep in range(body_reps):
                order_g = sorted(range(NT // GRP), key=lambda g: ptab[g * GRP])
                for g in order_g:
                    t0 = g * GRP
                    pt = ptab[t0]            # equal within a group
                    base = tilebase[t0]
                    ns = GRP * pt            # slots per partition in group
                    feat = fpool.tile([P, GRP * pmax * 24], mybir.dt.float16,
                                      tag="feat")
                    nc.sync.dma_start(
                        out=feat[:, :ns * 24].rearrange("p (t x) -> p t x", t=GRP),
                        in_=SF[base:base + GRP * P * pt, :].rearrange(
                            "(t p j) f -> p t (j f)", t=GRP, p=P),
                    )
                    fv = feat[:, :ns * 24].rearrange("p (t j f) -> p t j f",
                                                     t=GRP, f=24)
                    # s = a_src + a_dst (fp16 2x, in place)
                    nc.vector.tensor_tensor(
                        out=fv[:, :, :, 16:24],
                        in0=fv[:, :, :, 16:24],
                        in1=AD_sb[:, t0:t0 + GRP, None, :].broadcast_to(
                            [P, GRP, pt, HEADS]),
                        op=mybir.AluOpType.add)
                    # p = exp(lrelu(s))  -- both on the Scalar engine, in place
                    nc.scalar.activation(out=fv[:, :, :, 16:24],
                                         in_=fv[:, :, :, 16:24],
                                         func=mybir.ActivationFunctionType.Prelu,
                                         alpha=NEG_SLOPE)
                    nc.scalar.activation(out=fv[:, :, :, 16:24],
                                         in_=fv[:, :, :, 16:24],
                                         func=mybir.ActivationFunctionType.Exp,
                                         bias=shift_sb[:])
                    # q = p * h in place (h is c-major: [c=2, h=8])
                    nc.vector.tensor_tensor(
                        out=fv[:, :, :, 0:16].rearrange("p t j (c h) -> p t j c h",
                                                        c=2),
                        in0=fv[:, :, :, 0:16].rearrange("p t j (c h) -> p t j c h",
                                                        c=2),
                        in1=fv[:, :, :, None, 16:24].broadcast_to(
                            [P, GRP, pt, 2, HEADS]),
                        op=mybir.AluOpType.mult)
                    # pairwise fold tree over j: [q|p] summed per dst
                    cur = pt
                    while cur > 2:
                        k = cur // 2
                        nc.vector.tensor_tensor(
                            out=fv[:, :, 0:k, :],
                            in0=fv[:, :, 0:k, :],
                            in1=fv[:, :, cur - k:cur, :],
                            op=mybir.AluOpType.add)
                        cur -= k
                    if cur == 2:
                        nc.vector.tensor_tensor(
                            out=SQ[:, t0:t0 + GRP, :],
                            in0=fv[:, :, 0, :],
                            in1=fv[:, :, 1, :],
                            op=mybir.AluOpType.add)
                    else:  # pt == 1 tile group
                        nc.vector.tensor_copy(out=SQ[:, t0:t0 + GRP, :],
                                              in_=fv[:, :, 0, :])

            # ---- softmax normalization + bias + 16->2 head (fp32 tail)
            # Two halves: tiles [50:100] complete after the first 5 (small-pt,
            # high-tile-index) groups, so that half overlaps group compute.
            tmp = cpool.tile([P, NT, HC], mybir.dt.float32, tag="tmp")
            def _tail(lo, hi):
                n = hi - lo
                nc.vector.reciprocal(out=rec[:, lo:hi], in_=SQ[:, lo:hi, 16:24])
                nc.vector.tensor_tensor(
                    out=agg[:, lo:hi].rearrange("p t (c h) -> p t c h", c=2),
                    in0=SQ[:, lo:hi, 0:16].rearrange("p t (c h) -> p t c h", c=2),
                    in1=rec[:, lo:hi, None, :].broadcast_to([P, n, 2, HEADS]),
                    op=mybir.AluOpType.mult)
                nc.vector.tensor_tensor(
                    out=agg[:, lo:hi], in0=agg[:, lo:hi],
                    in1=brep_sb[:, None, :].broadcast_to([P, n, HC]),
                    op=mybir.AluOpType.add)
                for wsb, col in ((w0_sb, 0), (w1_sb, 1)):
                    nc.vector.tensor_tensor(
                        out=tmp[:, lo:hi], in0=agg[:, lo:hi],
                        in1=wsb[:, None, :].broadcast_to([P, n, HC]),
                        op=mybir.AluOpType.mult)
                    nc.vector.tensor_reduce(out=outb[:, lo:hi, col],
                                            in_=tmp[:, lo:hi],
                                            axis=mybir.AxisListType.X,
                                            op=mybir.AluOpType.add)
                nc.vector.tensor_tensor(
                    out=outb[:, lo:hi], in0=outb[:, lo:hi],
                    in1=bfc_sb[:, None, :].broadcast_to([P, n, 2]),
                    op=mybir.AluOpType.add)
                nc.sync.dma_start(
                    out=OUT.rearrange("p (t c) -> p t c", t=NT)[:, lo:hi],
                    in_=outb[:, lo:hi])
            _tail(NT // 2, NT)
            _tail(0, NT // 2)
    nc.compile()
    return nc


# ------------------------------------------------------------------ runner
class _Runner:
    """Reusable jitted shard_map executor for a compiled Bacc kernel."""

    def __init__(self, nc, in_maps):
        import jax
        from jax.sharding import Mesh, PartitionSpec, NamedSharding
        from jax.experimental.shard_map import shard_map
        from concourse import bass2jax, mybir

        bass2jax.install_neuronx_cc_hook()
        partition_name = (nc.partition_id_tensor.name
                          if nc.partition_id_tensor else None)
        in_names, out_names, out_avals, zero_outs = [], [], [], []
        for alloc in nc.m.functions[0].allocations:
            if not isinstance(alloc, mybir.MemoryLocationSet):
                continue
            name = alloc.memorylocations[0].name
            if alloc.kind == "ExternalInput":
                if name != partition_name:
                    in_names.append(name)
            elif alloc.kind == "ExternalOutput":
                shape = tuple(alloc.tensor_shape)
                dtype = mybir.dt.np(alloc.dtype)
                out_names.append(name)
                out_avals.append(jax.core.ShapedArray(shape, dtype))
                zero_outs.append(np.zeros(shape, dtype))
        n_params = len(in_names)
        all_in = list(in_names) + list(out_names)
        if partition_name is not None:
            all_in.append(partition_name)

        def _body(*args):
            operands = list(args)
            if partition_name is not None:
                operands.append(bass2jax.partition_id_tensor())
            return tuple(bass2jax._bass_exec_p.bind(
                *operands, out_avals=tuple(out_avals), in_names=tuple(all_in),
                out_names=tuple(out_names), lowering_input_output_aliases=(),
                sim_require_finite=True, sim_require_nnan=True, nc=nc))

        devices = jax.devices()[:N_CORES]
        mesh = Mesh(np.asarray(devices), ("core",))
        specs = (PartitionSpec("core"),)
        self._fn = jax.jit(
            shard_map(_body, mesh=mesh,
                      in_specs=specs * (n_params + len(out_avals)),
                      out_specs=specs * len(out_avals), check_rep=False),
            keep_unused=True)
        per_core = [[np.asarray(m[name]) for name in in_names] for m in in_maps]
        concat_in = [np.concatenate([per_core[c][i] for c in range(N_CORES)], axis=0)
                     for i in range(n_params)]
        concat_zero = [np.zeros((N_CORES * z.shape[0], *z.shape[1:]), z.dtype)
                       for z in zero_outs]
        sh = NamedSharding(mesh, PartitionSpec("core"))
        self._args = [jax.device_put(a, sh) for a in concat_in + concat_zero]
        self._out_names = out_names
        self._out_avals = out_avals
        self._jax = jax

    def run(self):
        outs = self._fn(*self._args)
        return [
            {name: np.asarray(outs[i]).reshape(N_CORES, *self._out_avals[i].shape)[c]
             for i, name in enumerate(self._out_names)}
            for c in range(N_CORES)
        ]

    def time(self, iters=8, warmup=2):
        for _ in range(warmup):
            self._jax.block_until_ready(self._fn(*self._args))
        walls = []
        for _ in range(iters):
            t0 = time.perf_counter()
            self._jax.block_until_ready(self._fn(*self._args))
            walls.append(time.perf_counter() - t0)
        return min(walls)


# ----------------------------------------------------------- input builders
_CMAJ = np.array([0, 2, 4, 6, 8, 10, 12, 14, 1, 3, 5, 7, 9, 11, 13, 15])


def _build_maps1(x, W, att_src, att_dst):
    """Per-core kernel-1 inputs.  All host work is index shuffling."""
    E = np.zeros((HC, 32), np.float32)
    E[_CMAJ, np.arange(16)] = 1.0                       # c-major identity
    for h in range(HEADS):
        for c in range(OUT_C):
            E[h * OUT_C + c, 16 + h] = att_src[h, c]    # placement, no math
            E[h * OUT_C + c, 24 + h] = att_dst[h, c]
    Wt16 = np.ascontiguousarray(W.T)                    # [16, 128]
    maps1 = []
    for c in range(N_CORES):
        xT = np.zeros((P, NP), np.float32)
        xT[:, :NODES_PER_CORE] = x[c * NODES_PER_CORE:(c + 1) * NODES_PER_CORE].T
        maps1.append({"xT": xT, "Wt16": Wt16, "E": E})
    return maps1


def _build_maps2(shards, F_full, bias_gat, W_fc, b_fc):
    """Per-core kernel-2 inputs from the kernel-1 output F_full [N, 32] bf16."""
    S = shards["S"]
    brep = np.tile(bias_gat[_CMAJ].reshape(1, HC), (P, 1)).astype(np.float32)
    w0 = np.tile(W_fc[_CMAJ, 0].reshape(1, HC), (P, 1)).astype(np.float32)
    w1 = np.tile(W_fc[_CMAJ, 1].reshape(1, HC), (P, 1)).astype(np.float32)
    bfcr = np.tile(b_fc.reshape(1, 2), (P, 1)).astype(np.float32)
    maps2 = []
    for c in range(N_CORES):
        ssrc = shards["slot_srcs"][c]
        perm = shards["perms"][c]
        SF = np.zeros((S, 24), F16)
        SF[:, 16:24] = F16(PAD_ASRC)
        real = ssrc >= 0
        SF[real] = F_full[ssrc[real], 0:24]
        AD = np.zeros((NP, HEADS), F16)
        pr = perm >= 0
        AD[pr] = F_full[perm[pr], 24:32]
        AD = np.ascontiguousarray(
            AD.reshape(NT, P, HEADS).transpose(1, 0, 2)).reshape(P, NT * HEADS)
        maps2.append({"SF": SF, "AD": AD, "brep": brep, "w0": w0, "w1": w1,
                      "bfc": bfcr})
    return maps2


def _gather_F(res1):
    """Assemble F_full [N_NODES, 32] bf16 from per-core F [32, NP] outputs."""
    parts = []
    for c in range(N_CORES):
        Ft = res1[c]["F"]                    # [32, NP] bf16
        parts.append(np.ascontiguousarray(Ft[:, :NODES_PER_CORE].T))
    return np.concatenate(parts, axis=0)


# ------------------------------------------------------------------- kernel
def kernel(**inputs):
    x = np.asarray(inputs["x"], np.float32)
    edge_index = np.asarray(inputs["edge_index"])
    W = np.asarray(inputs["W"], np.float32)
    att_src = np.asarray(inputs["att_src"], np.float32)
    att_dst = np.asarray(inputs["att_dst"], np.float32)
    bias_gat = np.asarray(inputs["bias_gat"], np.float32)
    W_fc = np.asarray(inputs["W_fc"], np.float32)
    b_fc = np.asarray(inputs["b_fc"], np.float32)
    # edge_attr intentionally ignored (GATConv built without edge_dim).

    shards = _build_shards(edge_index)

    def _run_retrying(build_nc, maps, attempts=3):
        last = None
        for i in range(attempts):
            try:
                return _Runner(build_nc(), maps).run()
            except Exception as e:  # transient device desync seen on this setup
                last = e
                time.sleep(2.0)
        raise last

    # ---- kernel 1: F_T = [h(c-major) | a_src | a_dst] per node, bf16
    maps1 = _build_maps1(x, W, att_src, att_dst)
    res1 = _run_retrying(_build_kernel1, maps1)
    F_full = _gather_F(res1)

    # ---- host shuffle: materialize per-core dst-CSR slot streams (bf16)
    maps2 = _build_maps2(shards, F_full, bias_gat, W_fc, b_fc)

    # ---- kernel 2: edge math + aggregation + head
    res2 = _run_retrying(
        lambda: _build_kernel2(shards["ptab"], shards["tilebase"], shards["S"]),
        maps2)

    out = np.zeros((N_NODES, 2), np.float32)
    for c in range(N_CORES):
        perm = shards["perms"][c]
        pr = perm >= 0
        o = res2[c]["OUT"].reshape(P, NT, 2).transpose(1, 0, 2).reshape(NP, 2)
        out[perm[pr]] = o[pr]
    return out
